# revision 32
# baseline (speedup 1.0000x reference)
"""Two-layer GCN (BongardGNN) on 8 Trainium2 NeuronCores.

This toolchain has no usable data-dependent-addressing primitive, so the
kernel is organised as three dense device launches with host-side,
index-only reshuffles between them (the host never does arithmetic on
tensor values — it only sorts/pads/gathers/duplicates by the static
edge_index):

  P1 (8 cores): dis = 1/sqrt(1+deg); q0 = dis*x -> bf16 (+ dis in bf16)
  host: gather q0 rows into degree-bucketed, feature-major CSR slots
        (self-loop included as slot 0, so no separate self term)
  P2 (8 cores): agg1 = sum_d mg1        [bf16 halving-tree adds]
                s1   = disf * agg1      [bf16]
                h1   = relu(Wblk1 s1 + b1)   [block-diag PE matmul]
                q2   = disq * (Wblk2 h1) -> bf16
  host: gather q2 rows into slots
  P3 (8 cores): out  = dis*(sum_d mg2) + b2

Performance structure (per core, memory-regime):
  * messages stream in bf16; nodes globally degree-sorted and dealt
    round-robin over (column, group, core) so per-column slot capacity
    = ceil2(max degree+1 in that 64-node row): ~3% padding
  * the slot reduce is a halving tree of CONTIGUOUS bf16 tensor_tensor
    adds (DVE packed mode, ~2 elem/cycle) over d-major slot planes —
    ~2x faster than the segmented tensor_reduce path; adjacent
    capacity-runs are merged when the padding cost is smaller than the
    per-instruction overhead
  * feature-major gather layout (partition = group*16+feature) means
    the reduce output lands directly in matmul-rhs layout: no PE
    transposes; one block-diagonal weight matmul serves 4 node groups.
"""

import os
import sys
import types

import numpy as np
import ml_dtypes
import concourse.bacc as bacc
import concourse.tile as tile
from concourse import mybir
from concourse.bass_utils import run_bass_kernel_spmd

F32 = mybir.dt.float32
BF16 = mybir.dt.bfloat16
NPBF = ml_dtypes.bfloat16

TRACE = bool(os.environ.get("GNN_TRACE"))
LAST_EXEC_NS = []


def _enable_tracing():
    """Register the axon NTFF profile hook (absent from this image's antenv)
    and stub out the slow artifact upload. Test-time only (GNN_TRACE=1)."""
    if "antenv.axon_hooks" not in sys.modules:
        mod = types.ModuleType("antenv.axon_hooks")
        state = {}
        mod.set_axon_ntff_profile_hook = lambda h: state.update(h=h)
        mod.get_axon_ntff_profile_hook = lambda: state.get("h")
        sys.modules["antenv.axon_hooks"] = mod
        import antenv

        antenv.axon_hooks = mod
        sys.path.insert(0, "/root/.axon_site")
        from trn_agent_boot.trn_boot import _ntff_profile_via_ctypes

        mod.set_axon_ntff_profile_hook(
            _ntff_profile_via_ctypes("/opt/axon/libaxon_pjrt.so")
        )
    import concourse.bass_utils as bu

    bu.upload_artifacts = lambda tmpdir: "skipped"


def _run(nc, in_maps, core_ids):
    if TRACE:
        _enable_tracing()
        res = run_bass_kernel_spmd(nc, in_maps, core_ids=core_ids, trace=True)
        LAST_EXEC_NS.append(res.exec_time_ns)
        return res
    return run_bass_kernel_spmd(nc, in_maps, core_ids=core_ids)


N = 200000
NCORES = 8
D0, D1, D2 = 16, 32, 2
CORE_IDS = list(range(NCORES))

# P1 grid: contiguous 25000-node slices, node = p*P1K + k per core
NPC1 = N // NCORES     # 25000
P1K = 196
P1PAD = 128 * P1K      # 25088

# P2 grid: partition p = g*16 + f (g = node group, f = feature);
# per core K2 node columns per group; global rank r = k*64 + g*8 + c
K2 = 3136              # 7 blocks of 448; 64*K2 = 200704 >= N
BLK = 448
NBLK = K2 // BLK
N2PAD = 64 * K2        # 200704

# P3 grid: partition p = node lane; rank r = k*1024 + c*128 + p
K3 = 196               # 1024*K3 = 200704 >= N


def _ceil2(a):
    return ((a + 1) // 2) * 2


def _runs(caps):
    """Maximal (k0, k1, cap) runs of equal capacity."""
    runs = []
    k0 = 0
    for k in range(1, len(caps) + 1):
        if k == len(caps) or caps[k] != caps[k0]:
            runs.append((k0, k, int(caps[k0])))
            k0 = k
    return runs


def _merged_runs(caps, max_extra=768):
    """Runs of equal cap, greedily merging a run into its (higher-cap)
    predecessor when the extra padded slots cost less than the saved
    per-instruction overhead."""
    runs = _runs(caps)
    out = [list(runs[0])]
    for k0, k1, v in runs[1:]:
        p0, p1, pv = out[-1]
        if (k1 - k0) * (pv - v) <= max_extra:
            out[-1][1] = k1
        else:
            out.append([k0, k1, v])
    return [(a, b, v) for a, b, v in out]


def _chunk_runs(runs, max_elems=5600):
    """Split long runs into <=max_elems pieces: finer DMA/tree pipeline
    quanta and smaller staging buffers."""
    out = []
    for k0, k1, v in runs:
        L = k1 - k0
        if L * v <= max_elems:
            out.append((k0, k1, v))
            continue
        n = -(-(L * v) // max_elems)
        step = -(-L // n)
        a = k0
        while a < k1:
            b = min(a + step, k1)
            out.append((a, b, v))
            a = b
    return out


def build_p1():
    """deg -> dis (f32+bf16) -> q0(bf16), 25088 nodes per core."""
    nc = bacc.Bacc("TRN2", target_bir_lowering=False, debug=False)
    xc = nc.dram_tensor("xc", [128, P1K * D0], F32, kind="ExternalInput")
    degc = nc.dram_tensor("degc", [128, P1K], F32, kind="ExternalInput")
    q0o = nc.dram_tensor("q0o", [128, P1K * D0], BF16, kind="ExternalOutput")
    diso = nc.dram_tensor("diso", [128, P1K], F32, kind="ExternalOutput")
    disbo = nc.dram_tensor("disbo", [128, P1K], BF16, kind="ExternalOutput")

    with tile.TileContext(nc) as tc:
        with tc.tile_pool(name="pool", bufs=2) as pool, tc.tile_pool(
            name="cpool", bufs=1
        ) as cpool:
            dis = cpool.tile([128, P1K], F32)
            nc.sync.dma_start(out=dis[:], in_=degc[:])
            nc.vector.tensor_scalar_add(dis[:], dis[:], 1.0)
            nc.scalar.activation(dis[:], dis[:], mybir.ActivationFunctionType.Sqrt)
            nc.vector.reciprocal(dis[:], dis[:])
            nc.sync.dma_start(out=diso[:], in_=dis[:])
            disb = cpool.tile([128, P1K], BF16)
            nc.vector.tensor_copy(out=disb[:], in_=dis[:])
            nc.sync.dma_start(out=disbo[:], in_=disb[:])
            NCH = 2
            KC = P1K // NCH
            for u in range(NCH):
                xt = pool.tile([128, KC * D0], F32, tag="xt")
                nc.sync.dma_start(
                    out=xt[:], in_=xc[:, u * KC * D0:(u + 1) * KC * D0]
                )
                q0t = pool.tile([128, KC * D0], BF16, tag="q0t")
                nc.vector.tensor_tensor(
                    out=q0t[:].rearrange("p (k f) -> p k f", f=D0),
                    in0=xt[:].rearrange("p (k f) -> p k f", f=D0),
                    in1=dis[:, u * KC:(u + 1) * KC]
                    .rearrange("p (k o) -> p k o", o=1)
                    .to_broadcast([128, KC, D0]),
                    op=mybir.AluOpType.mult,
                )
                nc.sync.dma_start(
                    out=q0o[:, u * KC * D0:(u + 1) * KC * D0], in_=q0t[:]
                )
    nc.compile()
    return nc


def build_p2(runs2):
    """Tree slot-reduce + both GCN matmuls, feature-major, per core.

    mg1 column layout, per run (k0, k1, v): d-major slot planes —
    column off_r + d*L + (k-k0) holds q0[slot[node(g,k), d], f] at
    partition g*16+f. Runs arrive pre-sorted largest-first so the last
    tree (the serial tail after the final DMA) is the smallest. The
    reduce is an in-place halving tree of contiguous bf16 adds (DVE
    packed mode); q2 = disq*(W2 h1) runs on scalar(copy)+gpsimd so the
    DVE stays on the tree.
    """
    offs = []
    off = 0
    for k0, k1, v in runs2:
        offs.append(off)
        off += (k1 - k0) * v
    smg = off
    mgt_max = max((k1 - k0) * v for k0, k1, v in runs2)

    nc = bacc.Bacc("TRN2", target_bir_lowering=False, debug=False)
    mg1 = nc.dram_tensor("mg1", [128, smg], BF16, kind="ExternalInput")
    disf = nc.dram_tensor("disf", [128, K2], BF16, kind="ExternalInput")
    disq = nc.dram_tensor("disq", [8, 2 * K2], BF16, kind="ExternalInput")
    w1d = nc.dram_tensor("w1d", [128, 128], F32, kind="ExternalInput")
    w2d = nc.dram_tensor("w2d", [128, 8], F32, kind="ExternalInput")
    b1d = nc.dram_tensor("b1d", [128, 1], F32, kind="ExternalInput")
    q2d = nc.dram_tensor("q2d", [8, 2 * K2], BF16, kind="ExternalOutput")

    with tile.TileContext(nc) as tc:
        with (
            tc.tile_pool(name="pool", bufs=6) as pool,
            tc.tile_pool(name="hpool", bufs=2) as hpool,
            tc.tile_pool(name="cpool", bufs=1) as cpool,
            tc.tile_pool(name="psum", bufs=2, space="PSUM") as psum,
        ):
            disfs = cpool.tile([128, K2], BF16)
            disqs = cpool.tile([8, 2 * K2], BF16)
            w1f = cpool.tile([128, 128], F32)
            w1b = cpool.tile([128, 128], BF16)
            w2f = cpool.tile([128, 8], F32)
            w2b = cpool.tile([128, 8], BF16)
            b1s = cpool.tile([128, 1], F32)
            aggb = cpool.tile([128, K2], BF16)
            s1b = cpool.tile([128, K2], BF16)

            # ---- slot reduce: in-place bf16 halving tree per run ----
            for ri, ((k0, k1, v), off) in enumerate(zip(runs2, offs)):
                L = k1 - k0
                cur = L * v
                mgt = pool.tile([128, mgt_max], BF16, tag="mgt")
                nc.sync.dma_start(out=mgt[:, :cur], in_=mg1[:, off:off + cur])
                if ri == 0:
                    # small constant loads queue right after the first
                    # (largest) message chunk
                    nc.sync.dma_start(out=disfs[:], in_=disf[:])
                    nc.sync.dma_start(out=disqs[:], in_=disq[:])
                    nc.sync.dma_start(out=w1f[:], in_=w1d[:])
                    nc.sync.dma_start(out=w2f[:], in_=w2d[:])
                    nc.sync.dma_start(out=b1s[:], in_=b1d[:])
                    nc.vector.tensor_copy(out=w1b[:], in_=w1f[:])
                    nc.vector.tensor_copy(out=w2b[:], in_=w2f[:])
                vc = v
                while vc > 2:
                    if vc % 2:
                        # fold the odd top plane into plane 0 first
                        nc.vector.tensor_tensor(
                            out=mgt[:, :L],
                            in0=mgt[:, :L],
                            in1=mgt[:, (vc - 1) * L:vc * L],
                            op=mybir.AluOpType.add,
                        )
                        vc -= 1
                    h = vc // 2
                    if h == 1:
                        break
                    nc.vector.tensor_tensor(
                        out=mgt[:, :h * L],
                        in0=mgt[:, :h * L],
                        in1=mgt[:, h * L:2 * h * L],
                        op=mybir.AluOpType.add,
                    )
                    vc = h
                nc.vector.tensor_tensor(
                    out=aggb[:, k0:k1],
                    in0=mgt[:, :L],
                    in1=mgt[:, L:2 * L],
                    op=mybir.AluOpType.add,
                )

            # ---- per-block: s1, block-diagonal matmuls, q2 ----
            # emit blocks in the order their covering runs complete, so
            # the in-order engine queues never stall on a late block
            def block_ready(blk):
                b0, b1 = blk * BLK, (blk + 1) * BLK
                return max(
                    ri
                    for ri, (k0, k1, _) in enumerate(runs2)
                    if k0 < b1 and k1 > b0
                )

            for blk in sorted(range(NBLK), key=block_ready):
                kb0, kb1 = blk * BLK, (blk + 1) * BLK
                nc.vector.tensor_tensor(
                    out=s1b[:, kb0:kb1],
                    in0=aggb[:, kb0:kb1],
                    in1=disfs[:, kb0:kb1],
                    op=mybir.AluOpType.mult,
                )
                for h in (0, 1):
                    ps1 = psum.tile([128, BLK], F32, tag="ps1")
                    nc.tensor.matmul(
                        out=ps1[:],
                        lhsT=w1b[64 * h:64 * h + 64, :],
                        rhs=s1b[64 * h:64 * h + 64, kb0:kb1],
                        start=True,
                        stop=True,
                    )
                    h1s = hpool.tile([128, BLK], BF16, tag="h1s")
                    nc.scalar.activation(
                        h1s[:],
                        ps1[:],
                        mybir.ActivationFunctionType.Relu,
                        bias=b1s[:],
                    )
                    ps2 = psum.tile([8, BLK], F32, tag="ps2")
                    nc.tensor.matmul(
                        out=ps2[:], lhsT=w2b[:], rhs=h1s[:], start=True, stop=True
                    )
                    q2c = hpool.tile([8, BLK], BF16, tag="q2c")
                    nc.scalar.activation(
                        q2c[:], ps2[:], mybir.ActivationFunctionType.Copy
                    )
                    q2s = hpool.tile([8, BLK], BF16, tag="q2s")
                    nc.gpsimd.tensor_tensor(
                        out=q2s[:],
                        in0=q2c[:],
                        in1=disqs[:, h * K2 + kb0:h * K2 + kb1],
                        op=mybir.AluOpType.mult,
                    )
                    nc.sync.dma_start(
                        out=q2d[:, h * K2 + kb0:h * K2 + kb1], in_=q2s[:]
                    )
    nc.compile()
    return nc


def build_p3(caps3):
    """out = dis*(sum_d mg2) + b2 per core (self-loop is slot 0)."""
    coloff = np.zeros(K3 + 1, np.int64)
    np.cumsum(caps3, out=coloff[1:])
    S3 = int(coloff[K3])

    nc = bacc.Bacc("TRN2", target_bir_lowering=False, debug=False)
    mg2 = nc.dram_tensor("mg2", [128, 2 * S3], BF16, kind="ExternalInput")
    disl3 = nc.dram_tensor("disl3", [128, K3], F32, kind="ExternalInput")
    b2r = nc.dram_tensor("b2r", [128, D2], F32, kind="ExternalInput")
    out3 = nc.dram_tensor("out3", [128, K3 * D2], F32, kind="ExternalOutput")

    with tile.TileContext(nc) as tc:
        with tc.tile_pool(name="pool", bufs=2) as pool, tc.tile_pool(
            name="cpool", bufs=1
        ) as cpool:
            disls = cpool.tile([128, K3], F32)
            nc.sync.dma_start(out=disls[:], in_=disl3[:])
            b2s = cpool.tile([128, D2], F32)
            nc.sync.dma_start(out=b2s[:], in_=b2r[:])
            NCH = 2
            KC = K3 // NCH
            for u in range(NCH):
                ku0, ku1 = u * KC, (u + 1) * KC
                c0, c1 = 2 * int(coloff[ku0]), 2 * int(coloff[ku1])
                mgt = pool.tile([128, c1 - c0], BF16, tag="mgt")
                nc.sync.dma_start(out=mgt[:], in_=mg2[:, c0:c1])
                agg = pool.tile([128, KC * D2], F32, tag="agg")
                for k0, k1, cap in _runs(caps3[ku0:ku1]):
                    o0 = 2 * int(coloff[ku0 + k0]) - c0
                    o1 = 2 * int(coloff[ku0 + k1]) - c0
                    nc.vector.tensor_reduce(
                        out=agg[:, k0 * D2:k1 * D2],
                        in_=mgt[:, o0:o1].rearrange(
                            "p (k f d) -> p k f d", f=D2, d=cap
                        ),
                        axis=mybir.AxisListType.X,
                        op=mybir.AluOpType.add,
                    )
                nc.vector.tensor_tensor(
                    out=agg[:].rearrange("p (k f) -> p k f", f=D2),
                    in0=agg[:].rearrange("p (k f) -> p k f", f=D2),
                    in1=disls[:, ku0:ku1]
                    .rearrange("p (k o) -> p k o", o=1)
                    .to_broadcast([128, KC, D2]),
                    op=mybir.AluOpType.mult,
                )
                nc.vector.tensor_tensor(
                    out=agg[:].rearrange("p (k f) -> p k f", f=D2),
                    in0=agg[:].rearrange("p (k f) -> p k f", f=D2),
                    in1=b2s[:]
                    .rearrange("p (o f) -> p o f", o=1)
                    .to_broadcast([128, KC, D2]),
                    op=mybir.AluOpType.add,
                )
                nc.sync.dma_start(
                    out=out3[:, ku0 * D2:ku1 * D2], in_=agg[:]
                )
    nc.compile()
    return nc


def kernel(x, edge_index, W1, b1, W2, b2):
    LAST_EXEC_NS.clear()
    x = np.asarray(x, np.float32)
    W1 = np.asarray(W1, np.float32)
    b1 = np.asarray(b1, np.float32)
    W2 = np.asarray(W2, np.float32)
    b2 = np.asarray(b2, np.float32)
    src = np.asarray(edge_index[0], np.int64)
    dst = np.asarray(edge_index[1], np.int64)

    # ---- host index prep: dst-sorted slot table (self first) ----
    deg = np.bincount(dst, minlength=N).astype(np.int64)
    capmax = _ceil2(int(deg.max()) + 1)
    order_e = np.argsort(dst, kind="stable")
    s_src = src[order_e]
    s_dst = dst[order_e]
    starts = np.zeros(N + 1, np.int64)
    np.cumsum(deg, out=starts[1:])
    slot = np.full((N + 1, capmax), N, np.int64)  # row N = sentinel
    slot[:N, 0] = np.arange(N)                   # self-loop slot
    pos = np.arange(len(s_src)) - starts[s_dst]
    slot[s_dst, pos + 1] = s_src

    onode = np.argsort(-deg, kind="stable")
    order_ext = np.concatenate([onode, np.full(N2PAD - N, N, np.int64)])
    deg_ext = np.concatenate([deg[onode] + 1, np.ones(N2PAD - N, np.int64)])
    caps2 = np.maximum(_ceil2(deg_ext[::64]), 2).astype(np.int64)    # [K2]
    caps3 = np.maximum(_ceil2(deg_ext[::1024]), 2).astype(np.int64)  # [K3]
    nodes2 = order_ext.reshape(K2, 8, NCORES)    # [k, g, c]
    nodes3 = order_ext.reshape(K3, NCORES, 128)  # [k, c, p]
    # biggest tree first => smallest serial tail after the last DMA
    runs2 = sorted(
        _chunk_runs(_merged_runs(caps2)), key=lambda r: -(r[1] - r[0]) * r[2]
    )
    runs3 = _runs(caps3)
    caps3m = caps3
    coloff3 = np.zeros(K3 + 1, np.int64)
    np.cumsum(caps3m, out=coloff3[1:])

    # ---- P1: dis + q0(bf16) on 8 cores ----
    p1 = build_p1()
    in1 = []
    for c in range(NCORES):
        xp = np.zeros((P1PAD, D0), np.float32)
        xp[:NPC1] = x[c * NPC1:(c + 1) * NPC1]
        dg = np.zeros(P1PAD, np.float32)
        dg[:NPC1] = deg[c * NPC1:(c + 1) * NPC1]
        in1.append(
            {
                "xc": np.ascontiguousarray(xp.reshape(128, P1K * D0)),
                "degc": np.ascontiguousarray(dg.reshape(128, P1K)),
            }
        )
    r1 = _run(p1, in1, core_ids=CORE_IDS).results
    q0ext = np.zeros((N + 1, D0), NPBF)
    dis = np.zeros(N + 1, np.float32)
    disb = np.zeros(N + 1, NPBF)
    for c in range(NCORES):
        sl = slice(c * NPC1, (c + 1) * NPC1)
        q0ext[sl] = np.asarray(r1[c]["q0o"]).reshape(P1PAD, D0)[:NPC1]
        dis[sl] = np.asarray(r1[c]["diso"]).reshape(P1PAD)[:NPC1]
        disb[sl] = np.asarray(r1[c]["disbo"]).reshape(P1PAD)[:NPC1]

    # ---- host join 1: d-major bucketed mg1 slots ----
    p2 = build_p2(runs2)
    smg = sum((k1 - k0) * v for k0, k1, v in runs2)
    w1blk = np.zeros((128, 128), np.float32)
    w2blk = np.zeros((128, 8), np.float32)
    for g in range(4):
        w1blk[16 * g:16 * g + 16, 32 * g:32 * g + 32] = W1
        w1blk[64 + 16 * g:64 + 16 * g + 16, 32 * g:32 * g + 32] = W1
        w2blk[32 * g:32 * g + 32, 2 * g:2 * g + 2] = W2
    b1blk = np.ascontiguousarray(np.tile(b1, 4).reshape(128, 1))
    in2 = []
    for c in range(NCORES):
        grid = nodes2[:, :, c].T  # [8, K2]
        mg1 = np.empty((128, smg), NPBF)
        off = 0
        for k0, k1, v in runs2:
            L = k1 - k0
            idx = slot[grid[:, k0:k1], :v]          # [8, L, v]
            g = q0ext[idx.transpose(0, 2, 1)]       # [8, v, L, 16]
            mg1[:, off:off + L * v] = g.transpose(0, 3, 1, 2).reshape(128, -1)
            off += L * v
        disg = disb[grid]  # [8, K2] bf16
        disfc = np.ascontiguousarray(
            np.repeat(disg[:, None, :], 16, axis=1)
        ).reshape(128, K2)
        # disq rows r = 2*g' + j, cols h*K2 + k hold dis(node(4h+g', k))
        disqc = np.ascontiguousarray(
            np.repeat(
                disb[grid].reshape(2, 4, 1, K2).transpose(1, 2, 0, 3), 2, axis=1
            ).reshape(8, 2 * K2)
        )
        in2.append(
            {
                "mg1": mg1,
                "disf": disfc,
                "disq": disqc,
                "w1d": w1blk,
                "w2d": w2blk,
                "b1d": b1blk,
            }
        )
    r2 = _run(p2, in2, core_ids=CORE_IDS).results
    q2ext = np.zeros((N + 1, D2), NPBF)
    for c in range(NCORES):
        vals = (
            np.asarray(r2[c]["q2d"])
            .reshape(4, 2, 2, K2)       # [g', j, h, k]
            .transpose(2, 0, 3, 1)      # [h, g', k, j]
            .reshape(8, K2, 2)
        )
        q2ext[nodes2[:, :, c].T] = vals  # [8, K2, 2]
    q2ext[N] = 0

    # ---- host join 2: mg2 slots ----
    p3 = build_p3(caps3m)
    S3 = int(coloff3[K3])
    b2r = np.ascontiguousarray(np.tile(b2.reshape(1, D2), (128, 1)))
    in3 = []
    for c in range(NCORES):
        grid = nodes3[:, c, :].T  # [128, K3]
        mg2 = np.empty((128, 2 * S3), NPBF)
        for k0, k1, cap in runs3:
            g = q2ext[slot[grid[:, k0:k1], :cap]]  # [128, L, cap, 2]
            mg2[:, 2 * coloff3[k0]:2 * coloff3[k1]] = g.transpose(
                0, 1, 3, 2
            ).reshape(128, -1)
        dislc = np.ascontiguousarray(dis[grid])
        in3.append({"mg2": mg2, "disl3": dislc, "b2r": b2r})
    r3 = _run(p3, in3, core_ids=CORE_IDS).results
    outfull = np.zeros((N + 1, D2), np.float32)
    for c in range(NCORES):
        outfull[nodes3[:, c, :].T] = np.asarray(r3[c]["out3"]).reshape(
            128, K3, D2
        )
    return np.ascontiguousarray(outfull[:N])


# revision 35
# speedup vs baseline: 1.1598x; 1.1598x over previous
"""Two-layer GCN (BongardGNN) on 8 Trainium2 NeuronCores.

This toolchain has no usable data-dependent-addressing primitive, so the
kernel is organised as three dense device launches with host-side,
index-only reshuffles between them (the host never does arithmetic on
tensor values — it only sorts/pads/gathers/duplicates by the static
edge_index):

  P1 (8 cores): dis = 1/sqrt(1+deg); q0 = dis*x -> bf16 (+ dis in bf16)
  host: gather q0 rows into degree-bucketed, feature-major CSR slots
        (self-loop included as slot 0, so no separate self term)
  P2 (8 cores): agg1 = sum_d mg1        [bf16 halving-tree adds]
                s1   = disf * agg1      [bf16]
                h1   = relu(Wblk1 s1 + b1)   [block-diag PE matmul]
                q2   = disq * (Wblk2 h1) -> bf16
  host: gather q2 rows into slots
  P3 (8 cores): out  = dis*(sum_d mg2) + b2

Performance structure (per core, memory-regime):
  * messages stream in bf16; nodes globally degree-sorted and dealt
    round-robin over (column, group, core) so per-column slot capacity
    = ceil2(max degree+1 in that 64-node row): ~3% padding
  * the slot reduce is a halving tree of CONTIGUOUS bf16 tensor_tensor
    adds (DVE packed mode, ~2 elem/cycle) over d-major slot planes —
    ~2x faster than the segmented tensor_reduce path; adjacent
    capacity-runs are merged when the padding cost is smaller than the
    per-instruction overhead
  * feature-major gather layout (partition = group*16+feature) means
    the reduce output lands directly in matmul-rhs layout: no PE
    transposes; one block-diagonal weight matmul serves 4 node groups.
"""

import os
import sys
import types

import numpy as np
import ml_dtypes
import concourse.bacc as bacc
import concourse.tile as tile
from concourse import mybir
from concourse.bass_utils import run_bass_kernel_spmd

F32 = mybir.dt.float32
BF16 = mybir.dt.bfloat16
NPBF = ml_dtypes.bfloat16

TRACE = bool(os.environ.get("GNN_TRACE"))
LAST_EXEC_NS = []


def _enable_tracing():
    """Register the axon NTFF profile hook (absent from this image's antenv)
    and stub out the slow artifact upload. Test-time only (GNN_TRACE=1)."""
    if "antenv.axon_hooks" not in sys.modules:
        mod = types.ModuleType("antenv.axon_hooks")
        state = {}
        mod.set_axon_ntff_profile_hook = lambda h: state.update(h=h)
        mod.get_axon_ntff_profile_hook = lambda: state.get("h")
        sys.modules["antenv.axon_hooks"] = mod
        import antenv

        antenv.axon_hooks = mod
        sys.path.insert(0, "/root/.axon_site")
        from trn_agent_boot.trn_boot import _ntff_profile_via_ctypes

        mod.set_axon_ntff_profile_hook(
            _ntff_profile_via_ctypes("/opt/axon/libaxon_pjrt.so")
        )
    import concourse.bass_utils as bu

    bu.upload_artifacts = lambda tmpdir: "skipped"


def _run(nc, in_maps, core_ids):
    if TRACE:
        _enable_tracing()
        res = run_bass_kernel_spmd(nc, in_maps, core_ids=core_ids, trace=True)
        LAST_EXEC_NS.append(res.exec_time_ns)
        return res
    return run_bass_kernel_spmd(nc, in_maps, core_ids=core_ids)


N = 200000
NCORES = 8
D0, D1, D2 = 16, 32, 2
CORE_IDS = list(range(NCORES))

# P1 grid: contiguous 25000-node slices, node = p*P1K + k per core
NPC1 = N // NCORES     # 25000
P1K = 196
P1PAD = 128 * P1K      # 25088

# P2 grid: partition p = g*16 + f (g = node group, f = feature);
# per core K2 node columns per group; global rank r = k*64 + g*8 + c
K2 = 3136              # 7 blocks of 448; 64*K2 = 200704 >= N
BLK = 448
NBLK = K2 // BLK
N2PAD = 64 * K2        # 200704

# P3 grid: partition p = node lane; rank r = k*1024 + c*128 + p
K3 = 196               # 1024*K3 = 200704 >= N


def _ceil2(a):
    return ((a + 1) // 2) * 2


def _runs(caps):
    """Maximal (k0, k1, cap) runs of equal capacity."""
    runs = []
    k0 = 0
    for k in range(1, len(caps) + 1):
        if k == len(caps) or caps[k] != caps[k0]:
            runs.append((k0, k, int(caps[k0])))
            k0 = k
    return runs


def _merged_runs(caps, max_extra=768):
    """Runs of equal cap, greedily merging a run into its (higher-cap)
    predecessor when the extra padded slots cost less than the saved
    per-instruction overhead."""
    runs = _runs(caps)
    out = [list(runs[0])]
    for k0, k1, v in runs[1:]:
        p0, p1, pv = out[-1]
        if (k1 - k0) * (pv - v) <= max_extra:
            out[-1][1] = k1
        else:
            out.append([k0, k1, v])
    return [(a, b, v) for a, b, v in out]


def build_p1():
    """deg -> dis (f32+bf16) -> q0(bf16), 25088 nodes per core."""
    nc = bacc.Bacc("TRN2", target_bir_lowering=False, debug=False)
    xc = nc.dram_tensor("xc", [128, P1K * D0], F32, kind="ExternalInput")
    degc = nc.dram_tensor("degc", [128, P1K], F32, kind="ExternalInput")
    q0o = nc.dram_tensor("q0o", [128, P1K * D0], BF16, kind="ExternalOutput")
    diso = nc.dram_tensor("diso", [128, P1K], F32, kind="ExternalOutput")
    disbo = nc.dram_tensor("disbo", [128, P1K], BF16, kind="ExternalOutput")

    with tile.TileContext(nc) as tc:
        with tc.tile_pool(name="pool", bufs=2) as pool, tc.tile_pool(
            name="cpool", bufs=1
        ) as cpool:
            dis = cpool.tile([128, P1K], F32)
            nc.sync.dma_start(out=dis[:], in_=degc[:])
            nc.vector.tensor_scalar_add(dis[:], dis[:], 1.0)
            nc.scalar.activation(dis[:], dis[:], mybir.ActivationFunctionType.Sqrt)
            nc.vector.reciprocal(dis[:], dis[:])
            nc.sync.dma_start(out=diso[:], in_=dis[:])
            disb = cpool.tile([128, P1K], BF16)
            nc.vector.tensor_copy(out=disb[:], in_=dis[:])
            nc.sync.dma_start(out=disbo[:], in_=disb[:])
            NCH = 2
            KC = P1K // NCH
            for u in range(NCH):
                xt = pool.tile([128, KC * D0], F32, tag="xt")
                nc.sync.dma_start(
                    out=xt[:], in_=xc[:, u * KC * D0:(u + 1) * KC * D0]
                )
                q0t = pool.tile([128, KC * D0], BF16, tag="q0t")
                nc.vector.tensor_tensor(
                    out=q0t[:].rearrange("p (k f) -> p k f", f=D0),
                    in0=xt[:].rearrange("p (k f) -> p k f", f=D0),
                    in1=dis[:, u * KC:(u + 1) * KC]
                    .rearrange("p (k o) -> p k o", o=1)
                    .to_broadcast([128, KC, D0]),
                    op=mybir.AluOpType.mult,
                )
                nc.sync.dma_start(
                    out=q0o[:, u * KC * D0:(u + 1) * KC * D0], in_=q0t[:]
                )
    nc.compile()
    return nc


def build_p2(runs2):
    """Tree slot-reduce + both GCN matmuls, feature-major, per core.

    mg1 column layout, per run (k0, k1, v): d-major slot planes —
    column off_r + d*L + (k-k0) holds q0[slot[node(g,k), d], f] at
    partition g*16+f. Runs arrive pre-sorted largest-first so the last
    tree (the serial tail after the final DMA) is the smallest. The
    reduce is an in-place halving tree of contiguous bf16 adds (DVE
    packed mode); q2 = disq*(W2 h1) runs on scalar(copy)+gpsimd so the
    DVE stays on the tree.
    """
    offs = []
    off = 0
    for k0, k1, v in runs2:
        offs.append(off)
        off += (k1 - k0) * v
    smg = off
    mgt_max = max((k1 - k0) * v for k0, k1, v in runs2)

    nc = bacc.Bacc("TRN2", target_bir_lowering=False, debug=False)
    mg1 = nc.dram_tensor("mg1", [128, smg], BF16, kind="ExternalInput")
    disf = nc.dram_tensor("disf", [128, K2], BF16, kind="ExternalInput")
    disq = nc.dram_tensor("disq", [8, 2 * K2], BF16, kind="ExternalInput")
    w1d = nc.dram_tensor("w1d", [128, 128], F32, kind="ExternalInput")
    w2d = nc.dram_tensor("w2d", [128, 8], F32, kind="ExternalInput")
    b1d = nc.dram_tensor("b1d", [128, 1], F32, kind="ExternalInput")
    q2d = nc.dram_tensor("q2d", [8, 2 * K2], BF16, kind="ExternalOutput")

    with tile.TileContext(nc) as tc:
        with (
            tc.tile_pool(name="pool", bufs=3) as pool,
            tc.tile_pool(name="hpool", bufs=2) as hpool,
            tc.tile_pool(name="cpool", bufs=1) as cpool,
            tc.tile_pool(name="psum", bufs=2, space="PSUM") as psum,
        ):
            disfs = cpool.tile([128, K2], BF16)
            disqs = cpool.tile([8, 2 * K2], BF16)
            w1f = cpool.tile([128, 128], F32)
            w1b = cpool.tile([128, 128], BF16)
            w2f = cpool.tile([128, 8], F32)
            w2b = cpool.tile([128, 8], BF16)
            b1s = cpool.tile([128, 1], F32)
            aggb = cpool.tile([128, K2], BF16)
            s1b = cpool.tile([128, K2], BF16)

            # ---- slot reduce: in-place bf16 halving tree per run ----
            for ri, ((k0, k1, v), off) in enumerate(zip(runs2, offs)):
                L = k1 - k0
                cur = L * v
                mgt = pool.tile([128, mgt_max], BF16, tag="mgt")
                nc.sync.dma_start(out=mgt[:, :cur], in_=mg1[:, off:off + cur])
                if ri == 0:
                    # small constant loads queue right after the first
                    # (largest) message chunk
                    nc.sync.dma_start(out=disfs[:], in_=disf[:])
                    nc.sync.dma_start(out=disqs[:], in_=disq[:])
                    nc.sync.dma_start(out=w1f[:], in_=w1d[:])
                    nc.sync.dma_start(out=w2f[:], in_=w2d[:])
                    nc.sync.dma_start(out=b1s[:], in_=b1d[:])
                    nc.vector.tensor_copy(out=w1b[:], in_=w1f[:])
                    nc.vector.tensor_copy(out=w2b[:], in_=w2f[:])
                vc = v
                while vc > 2:
                    if vc % 2:
                        # fold the odd top plane into plane 0 first
                        nc.vector.tensor_tensor(
                            out=mgt[:, :L],
                            in0=mgt[:, :L],
                            in1=mgt[:, (vc - 1) * L:vc * L],
                            op=mybir.AluOpType.add,
                        )
                        vc -= 1
                    h = vc // 2
                    if h == 1:
                        break
                    nc.vector.tensor_tensor(
                        out=mgt[:, :h * L],
                        in0=mgt[:, :h * L],
                        in1=mgt[:, h * L:2 * h * L],
                        op=mybir.AluOpType.add,
                    )
                    vc = h
                nc.vector.tensor_tensor(
                    out=aggb[:, k0:k1],
                    in0=mgt[:, :L],
                    in1=mgt[:, L:2 * L],
                    op=mybir.AluOpType.add,
                )

            # ---- per-block: s1, block-diagonal matmuls, q2 ----
            # emit blocks in the order their covering runs complete, so
            # the in-order engine queues never stall on a late block
            def block_ready(blk):
                b0, b1 = blk * BLK, (blk + 1) * BLK
                return max(
                    ri
                    for ri, (k0, k1, _) in enumerate(runs2)
                    if k0 < b1 and k1 > b0
                )

            for blk in sorted(range(NBLK), key=block_ready):
                kb0, kb1 = blk * BLK, (blk + 1) * BLK
                nc.vector.tensor_tensor(
                    out=s1b[:, kb0:kb1],
                    in0=aggb[:, kb0:kb1],
                    in1=disfs[:, kb0:kb1],
                    op=mybir.AluOpType.mult,
                )
                for h in (0, 1):
                    ps1 = psum.tile([128, BLK], F32, tag="ps1")
                    nc.tensor.matmul(
                        out=ps1[:],
                        lhsT=w1b[64 * h:64 * h + 64, :],
                        rhs=s1b[64 * h:64 * h + 64, kb0:kb1],
                        start=True,
                        stop=True,
                    )
                    h1s = hpool.tile([128, BLK], BF16, tag="h1s")
                    nc.scalar.activation(
                        h1s[:],
                        ps1[:],
                        mybir.ActivationFunctionType.Relu,
                        bias=b1s[:],
                    )
                    ps2 = psum.tile([8, BLK], F32, tag="ps2")
                    nc.tensor.matmul(
                        out=ps2[:], lhsT=w2b[:], rhs=h1s[:], start=True, stop=True
                    )
                    q2c = hpool.tile([8, BLK], BF16, tag="q2c")
                    nc.scalar.activation(
                        q2c[:], ps2[:], mybir.ActivationFunctionType.Copy
                    )
                    q2s = hpool.tile([8, BLK], BF16, tag="q2s")
                    nc.gpsimd.tensor_tensor(
                        out=q2s[:],
                        in0=q2c[:],
                        in1=disqs[:, h * K2 + kb0:h * K2 + kb1],
                        op=mybir.AluOpType.mult,
                    )
                    nc.sync.dma_start(
                        out=q2d[:, h * K2 + kb0:h * K2 + kb1], in_=q2s[:]
                    )
    nc.compile()
    return nc


def build_p3(caps3):
    """out = dis*(sum_d mg2) + b2 per core (self-loop is slot 0)."""
    coloff = np.zeros(K3 + 1, np.int64)
    np.cumsum(caps3, out=coloff[1:])
    S3 = int(coloff[K3])

    nc = bacc.Bacc("TRN2", target_bir_lowering=False, debug=False)
    mg2 = nc.dram_tensor("mg2", [128, 2 * S3], BF16, kind="ExternalInput")
    disl3 = nc.dram_tensor("disl3", [128, K3], F32, kind="ExternalInput")
    b2r = nc.dram_tensor("b2r", [128, D2], F32, kind="ExternalInput")
    out3 = nc.dram_tensor("out3", [128, K3 * D2], F32, kind="ExternalOutput")

    with tile.TileContext(nc) as tc:
        with tc.tile_pool(name="pool", bufs=2) as pool, tc.tile_pool(
            name="cpool", bufs=1
        ) as cpool:
            disls = cpool.tile([128, K3], F32)
            nc.sync.dma_start(out=disls[:], in_=disl3[:])
            b2s = cpool.tile([128, D2], F32)
            nc.sync.dma_start(out=b2s[:], in_=b2r[:])
            NCH = 2
            KC = K3 // NCH
            for u in range(NCH):
                ku0, ku1 = u * KC, (u + 1) * KC
                c0, c1 = 2 * int(coloff[ku0]), 2 * int(coloff[ku1])
                mgt = pool.tile([128, c1 - c0], BF16, tag="mgt")
                nc.sync.dma_start(out=mgt[:], in_=mg2[:, c0:c1])
                agg = pool.tile([128, KC * D2], F32, tag="agg")
                for k0, k1, cap in _runs(caps3[ku0:ku1]):
                    o0 = 2 * int(coloff[ku0 + k0]) - c0
                    o1 = 2 * int(coloff[ku0 + k1]) - c0
                    nc.vector.tensor_reduce(
                        out=agg[:, k0 * D2:k1 * D2],
                        in_=mgt[:, o0:o1].rearrange(
                            "p (k f d) -> p k f d", f=D2, d=cap
                        ),
                        axis=mybir.AxisListType.X,
                        op=mybir.AluOpType.add,
                    )
                nc.vector.tensor_tensor(
                    out=agg[:].rearrange("p (k f) -> p k f", f=D2),
                    in0=agg[:].rearrange("p (k f) -> p k f", f=D2),
                    in1=disls[:, ku0:ku1]
                    .rearrange("p (k o) -> p k o", o=1)
                    .to_broadcast([128, KC, D2]),
                    op=mybir.AluOpType.mult,
                )
                nc.vector.tensor_tensor(
                    out=agg[:].rearrange("p (k f) -> p k f", f=D2),
                    in0=agg[:].rearrange("p (k f) -> p k f", f=D2),
                    in1=b2s[:]
                    .rearrange("p (o f) -> p o f", o=1)
                    .to_broadcast([128, KC, D2]),
                    op=mybir.AluOpType.add,
                )
                nc.sync.dma_start(
                    out=out3[:, ku0 * D2:ku1 * D2], in_=agg[:]
                )
    nc.compile()
    return nc


def kernel(x, edge_index, W1, b1, W2, b2):
    LAST_EXEC_NS.clear()
    x = np.asarray(x, np.float32)
    W1 = np.asarray(W1, np.float32)
    b1 = np.asarray(b1, np.float32)
    W2 = np.asarray(W2, np.float32)
    b2 = np.asarray(b2, np.float32)
    src = np.asarray(edge_index[0], np.int64)
    dst = np.asarray(edge_index[1], np.int64)

    # ---- host index prep: dst-sorted slot table (self first) ----
    deg = np.bincount(dst, minlength=N).astype(np.int64)
    capmax = _ceil2(int(deg.max()) + 1)
    order_e = np.argsort(dst, kind="stable")
    s_src = src[order_e]
    s_dst = dst[order_e]
    starts = np.zeros(N + 1, np.int64)
    np.cumsum(deg, out=starts[1:])
    slot = np.full((N + 1, capmax), N, np.int64)  # row N = sentinel
    slot[:N, 0] = np.arange(N)                   # self-loop slot
    pos = np.arange(len(s_src)) - starts[s_dst]
    slot[s_dst, pos + 1] = s_src

    onode = np.argsort(-deg, kind="stable")
    order_ext = np.concatenate([onode, np.full(N2PAD - N, N, np.int64)])
    deg_ext = np.concatenate([deg[onode] + 1, np.ones(N2PAD - N, np.int64)])
    caps2 = np.maximum(_ceil2(deg_ext[::64]), 2).astype(np.int64)    # [K2]
    caps3 = np.maximum(_ceil2(deg_ext[::1024]), 2).astype(np.int64)  # [K3]
    nodes2 = order_ext.reshape(K2, 8, NCORES)    # [k, g, c]
    nodes3 = order_ext.reshape(K3, NCORES, 128)  # [k, c, p]
    # biggest tree first => smallest serial tail after the last DMA
    runs2 = sorted(
        _merged_runs(caps2), key=lambda r: -(r[1] - r[0]) * r[2]
    )
    runs3 = _runs(caps3)
    caps3m = caps3
    coloff3 = np.zeros(K3 + 1, np.int64)
    np.cumsum(caps3m, out=coloff3[1:])

    # ---- P1: dis + q0(bf16) on 8 cores ----
    p1 = build_p1()
    in1 = []
    for c in range(NCORES):
        xp = np.zeros((P1PAD, D0), np.float32)
        xp[:NPC1] = x[c * NPC1:(c + 1) * NPC1]
        dg = np.zeros(P1PAD, np.float32)
        dg[:NPC1] = deg[c * NPC1:(c + 1) * NPC1]
        in1.append(
            {
                "xc": np.ascontiguousarray(xp.reshape(128, P1K * D0)),
                "degc": np.ascontiguousarray(dg.reshape(128, P1K)),
            }
        )
    r1 = _run(p1, in1, core_ids=CORE_IDS).results
    q0ext = np.zeros((N + 1, D0), NPBF)
    dis = np.zeros(N + 1, np.float32)
    disb = np.zeros(N + 1, NPBF)
    for c in range(NCORES):
        sl = slice(c * NPC1, (c + 1) * NPC1)
        q0ext[sl] = np.asarray(r1[c]["q0o"]).reshape(P1PAD, D0)[:NPC1]
        dis[sl] = np.asarray(r1[c]["diso"]).reshape(P1PAD)[:NPC1]
        disb[sl] = np.asarray(r1[c]["disbo"]).reshape(P1PAD)[:NPC1]

    # ---- host join 1: d-major bucketed mg1 slots ----
    p2 = build_p2(runs2)
    smg = sum((k1 - k0) * v for k0, k1, v in runs2)
    w1blk = np.zeros((128, 128), np.float32)
    w2blk = np.zeros((128, 8), np.float32)
    for g in range(4):
        w1blk[16 * g:16 * g + 16, 32 * g:32 * g + 32] = W1
        w1blk[64 + 16 * g:64 + 16 * g + 16, 32 * g:32 * g + 32] = W1
        w2blk[32 * g:32 * g + 32, 2 * g:2 * g + 2] = W2
    b1blk = np.ascontiguousarray(np.tile(b1, 4).reshape(128, 1))
    in2 = []
    for c in range(NCORES):
        grid = nodes2[:, :, c].T  # [8, K2]
        mg1 = np.empty((128, smg), NPBF)
        off = 0
        for k0, k1, v in runs2:
            L = k1 - k0
            idx = slot[grid[:, k0:k1], :v]          # [8, L, v]
            g = q0ext[idx.transpose(0, 2, 1)]       # [8, v, L, 16]
            mg1[:, off:off + L * v] = g.transpose(0, 3, 1, 2).reshape(128, -1)
            off += L * v
        disg = disb[grid]  # [8, K2] bf16
        disfc = np.ascontiguousarray(
            np.repeat(disg[:, None, :], 16, axis=1)
        ).reshape(128, K2)
        # disq rows r = 2*g' + j, cols h*K2 + k hold dis(node(4h+g', k))
        disqc = np.ascontiguousarray(
            np.repeat(
                disb[grid].reshape(2, 4, 1, K2).transpose(1, 2, 0, 3), 2, axis=1
            ).reshape(8, 2 * K2)
        )
        in2.append(
            {
                "mg1": mg1,
                "disf": disfc,
                "disq": disqc,
                "w1d": w1blk,
                "w2d": w2blk,
                "b1d": b1blk,
            }
        )
    r2 = _run(p2, in2, core_ids=CORE_IDS).results
    q2ext = np.zeros((N + 1, D2), NPBF)
    for c in range(NCORES):
        vals = (
            np.asarray(r2[c]["q2d"])
            .reshape(4, 2, 2, K2)       # [g', j, h, k]
            .transpose(2, 0, 3, 1)      # [h, g', k, j]
            .reshape(8, K2, 2)
        )
        q2ext[nodes2[:, :, c].T] = vals  # [8, K2, 2]
    q2ext[N] = 0

    # ---- host join 2: mg2 slots ----
    p3 = build_p3(caps3m)
    S3 = int(coloff3[K3])
    b2r = np.ascontiguousarray(np.tile(b2.reshape(1, D2), (128, 1)))
    in3 = []
    for c in range(NCORES):
        grid = nodes3[:, c, :].T  # [128, K3]
        mg2 = np.empty((128, 2 * S3), NPBF)
        for k0, k1, cap in runs3:
            g = q2ext[slot[grid[:, k0:k1], :cap]]  # [128, L, cap, 2]
            mg2[:, 2 * coloff3[k0]:2 * coloff3[k1]] = g.transpose(
                0, 1, 3, 2
            ).reshape(128, -1)
        dislc = np.ascontiguousarray(dis[grid])
        in3.append({"mg2": mg2, "disl3": dislc, "b2r": b2r})
    r3 = _run(p3, in3, core_ids=CORE_IDS).results
    outfull = np.zeros((N + 1, D2), np.float32)
    for c in range(NCORES):
        outfull[nodes3[:, c, :].T] = np.asarray(r3[c]["out3"]).reshape(
            128, K3, D2
        )
    return np.ascontiguousarray(outfull[:N])
